# revision 5
# baseline (speedup 1.0000x reference)
"""3x3 median blur on Trainium2, data-parallel across 8 NeuronCores.

Input:  image (16, 3, 512, 512) float32
Output: median-blur(3x3, zero-padded) same shape.

Strategy:
- Shard batch across 8 cores: core c handles images [2c, 2c+2) -> 6 channel
  planes of 512x512 each.
- Host pads each plane to 514x514 with zeros (matches zero-pad semantics and
  removes all border special-casing on device).
- Device kernel (per core): 3 passes; each pass handles 2 planes. 128
  partitions x (64 row-chunks of 8 output rows per plane). Each partition
  holds a 10-row x 514-col slab (8 output rows + 1 halo row each side).
- Exact fp32 median-of-9 via separable sorting network on the vector engine:
    vertical: sort each 3-tall column into (lo, mid, hi) using shared
      adjacent-row min/max pairs;
    horizontal: median9 = med3(max3(lo), med3(mid), min3(hi)) with shared
      even/odd column pairs.
  ~15 tensor_tensor min/max ops per output pixel, all fp32-exact.
"""

import sys

if "/opt/trn_rl_repo" not in sys.path:
    sys.path.insert(0, "/opt/trn_rl_repo")

import numpy as np

import concourse.bass as bass
import concourse.tile as tile
from concourse import bacc, mybir
from concourse.bass_utils import run_bass_kernel_spmd

F32 = mybir.dt.float32
MAX = mybir.AluOpType.max
MIN = mybir.AluOpType.min

N_CORES = 8
B, C, H, W = 16, 3, 512, 512
PLANES = (B * C) // N_CORES  # 6 planes per core
PH, PW = H + 2, W + 2  # 514, 514
PLANE = PH * PW  # padded plane elems
OPLANE = H * W  # output plane elems

N_PASSES = PLANES // 2  # 2 planes per pass
CHUNK = 8  # output rows per partition per pass
SLAB = CHUNK + 2  # input rows per partition slab
SUB = 4  # output rows per sub-pass (2 sub-passes per pass)

_CACHED = {}


def _ap(apref, off, dims):
    """View into a tile/dram AP with explicit [step, num] free dims."""
    part = list(apref.ap[0])
    return bass.AP(apref.tensor, apref.offset + off, [part] + [list(d) for d in dims])


def _dram(handle, off, dims):
    return bass.AP(handle, off, [list(d) for d in dims])


def _build():
    nc = bacc.Bacc(
        "TRN2", target_bir_lowering=False, debug=False, num_devices=N_CORES
    )
    xin = nc.dram_tensor("xpad", [PLANES, PH, PW], F32, kind="ExternalInput")
    yout = nc.dram_tensor("y", [PLANES, H, W], F32, kind="ExternalOutput")

    with tile.TileContext(nc) as tc:
        _body(tc, nc, xin, yout)

    nc.compile()
    return nc


def _body(tc, nc, xin, yout):
    from contextlib import ExitStack

    ctx = ExitStack()
    with ctx:
        xpool = ctx.enter_context(tc.tile_pool(name="xpool", bufs=2))
        vpool = ctx.enter_context(tc.tile_pool(name="vpool", bufs=1))
        lmh = ctx.enter_context(tc.tile_pool(name="lmh", bufs=1))
        hpool = ctx.enter_context(tc.tile_pool(name="hpool", bufs=1))
        abc = ctx.enter_context(tc.tile_pool(name="abc", bufs=1))
        fin = ctx.enter_context(tc.tile_pool(name="fin", bufs=1))
        opool = ctx.enter_context(tc.tile_pool(name="opool", bufs=2))

        tt = nc.vector.tensor_tensor

        for t in range(N_PASSES):
            X = xpool.tile([128, SLAB * PW], F32, name="X")
            # input DMA: partition p = 64*h + c  <- plane 2t+h, rows 8c..8c+10.
            # Chunked (rows 0..6, rows 6..10) so the first sub-pass can start
            # computing while the rest of the slab is still in flight; one
            # 128-partition DMA per chunk (src dims [2 planes, 64 chunks, :]).
            for r0, r1 in ((0, 6), (6, SLAB)):
                nc.sync.dma_start(
                    X[:, r0 * PW : r1 * PW],
                    _dram(
                        xin,
                        2 * t * PLANE + r0 * PW,
                        [[PLANE, 2], [CHUNK * PW, 64], [1, (r1 - r0) * PW]],
                    ),
                )

            for sp in range(2):
                b = sp * SUB  # slab row base for this sub-pass
                # X row views (within the slab): step is rows of PW elems
                def xv(r0, nrows, rstep=2):
                    return _ap(X, (b + r0) * PW, [[rstep * PW, nrows], [1, PW]])

                # ---- vertical stage: column sort3 -> lo, mid, hi ----
                # pairs at slab rows (b+1,b+2) and (b+3,b+4)
                pmin = vpool.tile([128, 2 * PW], F32, name="pmin")
                pmax = vpool.tile([128, 2 * PW], F32, name="pmax")
                pv = [[PW, 2], [1, PW]]
                tt(_ap(pmin, 0, pv), xv(1, 2), xv(2, 2), MIN)
                tt(_ap(pmax, 0, pv), xv(1, 2), xv(2, 2), MAX)

                lo = lmh.tile([128, SUB * PW], F32, name="lo")
                mid = lmh.tile([128, SUB * PW], F32, name="mid")
                hi = lmh.tile([128, SUB * PW], F32, name="hi")
                uo = vpool.tile([128, 2 * PW], F32, name="uo")
                ue = vpool.tile([128, 2 * PW], F32, name="ue")

                # tile rows r=0..3 <-> output slab rows s=b+1+r
                def lv(tl, r0):  # rows {r0, r0+2} of a SUB-row tile
                    return _ap(tl, r0 * PW, [[2 * PW, 2], [1, PW]])

                pm = _ap(pmin, 0, pv)
                pM = _ap(pmax, 0, pv)
                # odd outputs (tile rows 0,2): third element = X[s-1] = xv(0,2)
                # even outputs (tile rows 1,3): third = X[s+1] = xv(3,2)
                tt(lv(lo, 0), pm, xv(0, 2), MIN)
                tt(lv(lo, 1), pm, xv(3, 2), MIN)
                tt(lv(hi, 0), pM, xv(0, 2), MAX)
                tt(lv(hi, 1), pM, xv(3, 2), MAX)
                tt(_ap(uo, 0, pv), pM, xv(0, 2), MIN)
                tt(_ap(ue, 0, pv), pM, xv(3, 2), MIN)
                tt(lv(mid, 0), pm, _ap(uo, 0, pv), MAX)
                tt(lv(mid, 1), pm, _ap(ue, 0, pv), MAX)

                # ---- horizontal stage ----
                # even/odd column pairs over 514 cols -> 257 pairs
                NP = PW // 2  # 257
                def cview(tl, c0, ncols, cstep=2):
                    return _ap(tl, c0, [[PW, SUB], [cstep, ncols]])

                def pview(tl, k0, nk):
                    return _ap(tl, k0, [[NP, SUB], [1, nk]])

                mlo = hpool.tile([128, SUB * NP], F32, name="mlo")
                mhi = hpool.tile([128, SUB * NP], F32, name="mhi")
                pmn = hpool.tile([128, SUB * NP], F32, name="pmn")
                pmx = hpool.tile([128, SUB * NP], F32, name="pmx")

                tt(pview(mlo, 0, NP), cview(lo, 0, NP), cview(lo, 1, NP), MAX)
                tt(pview(mhi, 0, NP), cview(hi, 0, NP), cview(hi, 1, NP), MIN)
                tt(pview(pmn, 0, NP), cview(mid, 0, NP), cview(mid, 1, NP), MIN)
                tt(pview(pmx, 0, NP), cview(mid, 0, NP), cview(mid, 1, NP), MAX)

                # output-column views of W-wide tiles (row stride W)
                def ov(tl, c0, ncols, cstep=2):
                    return _ap(tl, c0, [[W, SUB], [cstep, ncols]])

                A = abc.tile([128, SUB * W], F32, name="A")
                Bt = abc.tile([128, SUB * W], F32, name="Bt")
                Ct = abc.tile([128, SUB * W], F32, name="Ct")
                ube = hpool.tile([128, SUB * (W // 2)], F32, name="ube")
                ubo = hpool.tile([128, SUB * (W // 2)], F32, name="ubo")
                NH = W // 2  # 256

                def uv(tl):
                    return _ap(tl, 0, [[NH, SUB], [1, NH]])

                # A = sliding max3 of lo; out col j (0-based output coords)
                # j even: max(mlo[j/2], lo[j+2]); j odd: max(mlo[(j+1)/2], lo[j])
                tt(ov(A, 0, NH), pview(mlo, 0, NH), cview(lo, 2, NH), MAX)
                tt(ov(A, 1, NH), pview(mlo, 1, NH), cview(lo, 1, NH), MAX)
                # C = sliding min3 of hi
                tt(ov(Ct, 0, NH), pview(mhi, 0, NH), cview(hi, 2, NH), MIN)
                tt(ov(Ct, 1, NH), pview(mhi, 1, NH), cview(hi, 1, NH), MIN)
                # B = sliding med3 of mid: med3(a, pair) = max(pmn, min(a, pmx))
                tt(uv(ube), cview(mid, 2, NH), pview(pmx, 0, NH), MIN)
                tt(ov(Bt, 0, NH), pview(pmn, 0, NH), uv(ube), MAX)
                tt(uv(ubo), cview(mid, 1, NH), pview(pmx, 1, NH), MIN)
                tt(ov(Bt, 1, NH), pview(pmn, 1, NH), uv(ubo), MAX)

                # ---- final med3(A, B, C) ----
                flat = [[1, SUB * W]]
                mn = fin.tile([128, SUB * W], F32, name="mn")
                mx = fin.tile([128, SUB * W], F32, name="mx")
                t2 = fin.tile([128, SUB * W], F32, name="t2")
                res = opool.tile([128, SUB * W], F32, name="res")
                tt(_ap(mn, 0, flat), _ap(A, 0, flat), _ap(Bt, 0, flat), MIN)
                tt(_ap(mx, 0, flat), _ap(A, 0, flat), _ap(Bt, 0, flat), MAX)
                tt(_ap(t2, 0, flat), _ap(mx, 0, flat), _ap(Ct, 0, flat), MIN)
                tt(_ap(res, 0, flat), _ap(mn, 0, flat), _ap(t2, 0, flat), MAX)

                # output DMA: partition p -> plane, rows 8*(p%64)+4*sp..+4
                for h in range(2):
                    nc.sync.dma_start(
                        _dram(
                            yout,
                            (2 * t + h) * OPLANE + sp * SUB * W,
                            [[CHUNK * W, 64], [W, SUB], [1, W]],
                        ),
                        res[64 * h : 64 * h + 64, :],
                    )


def _get_nc():
    if "nc" not in _CACHED:
        _CACHED["nc"] = _build()
    return _CACHED["nc"]


def kernel(image: np.ndarray, _trace: bool = False):
    assert image.shape == (B, C, H, W) and image.dtype == np.float32
    nc = _get_nc()

    in_maps = []
    per_core = B // N_CORES
    for c in range(N_CORES):
        shard = image[c * per_core : (c + 1) * per_core].reshape(PLANES, H, W)
        padded = np.zeros((PLANES, PH, PW), dtype=np.float32)
        padded[:, 1:-1, 1:-1] = shard
        in_maps.append({"xpad": padded})

    res = run_bass_kernel_spmd(
        nc, in_maps, list(range(N_CORES)), trace=_trace
    )
    _CACHED["last_exec_ns"] = res.exec_time_ns

    out = np.empty((B, C, H, W), dtype=np.float32)
    for c in range(N_CORES):
        out[c * per_core : (c + 1) * per_core] = res.results[c]["y"].reshape(
            per_core, C, H, W
        )
    return out


# revision 6
# speedup vs baseline: 1.1222x; 1.1222x over previous
"""3x3 median blur on Trainium2, data-parallel across 8 NeuronCores.

Input:  image (16, 3, 512, 512) float32
Output: median-blur(3x3, zero-padded) same shape.

Strategy:
- Shard batch across 8 cores: core c handles images [2c, 2c+2) -> 6 channel
  planes of 512x512 each.
- Host pads each plane to 514x514 with zeros and restages it into device
  layout [pass][partition][slab]: per pass two planes, each split into 64
  row-chunks of 8 output rows; partition p holds a 10-row x 514-col slab
  (8 output rows + 1 halo row each side). Restaging makes every DMA a plain
  [128 partitions x contiguous] transfer at full SDMA rate.
- Device kernel (per core): 3 passes x 2 sub-passes; exact fp32 median-of-9
  via separable sorting network on the vector engine:
    vertical: sort each 3-tall column into (lo, mid, hi) using shared
      adjacent-row min/max pairs;
    horizontal: median9 = med3(max3(lo), med3(mid), min3(hi)) with shared
      even/odd column pairs.
  ~15 tensor_tensor min/max ops per output pixel, all fp32-exact.
- Output staged as [pass][sub-pass][partition][4*512]; host scatters back.
"""

import sys

if "/opt/trn_rl_repo" not in sys.path:
    sys.path.insert(0, "/opt/trn_rl_repo")

import numpy as np

import concourse.bass as bass
import concourse.tile as tile
from concourse import bacc, mybir
from concourse.bass_utils import run_bass_kernel_spmd

F32 = mybir.dt.float32
MAX = mybir.AluOpType.max
MIN = mybir.AluOpType.min

N_CORES = 8
B, C, H, W = 16, 3, 512, 512
PLANES = (B * C) // N_CORES  # 6 planes per core
PH, PW = H + 2, W + 2  # 514, 514

N_PASSES = PLANES // 2  # 2 planes per pass
CHUNK = 8  # output rows per partition per pass
SLAB = CHUNK + 2  # input rows per partition slab
SUB = 4  # output rows per sub-pass (2 sub-passes per pass)
SLABE = SLAB * PW  # slab elems per partition (5140)
OUTE = SUB * W  # output elems per partition per sub-pass (2048)

_CACHED = {}


def _ap(apref, off, dims):
    """View into a tile AP with explicit [step, num] free dims."""
    part = list(apref.ap[0])
    return bass.AP(apref.tensor, apref.offset + off, [part] + [list(d) for d in dims])


def _dram(handle, off, dims):
    return bass.AP(handle, off, [list(d) for d in dims])


def _build():
    nc = bacc.Bacc(
        "TRN2", target_bir_lowering=False, debug=False, num_devices=N_CORES
    )
    xin = nc.dram_tensor(
        "xs", [N_PASSES, 128, SLABE], F32, kind="ExternalInput"
    )
    yout = nc.dram_tensor(
        "ys", [N_PASSES, 2, 128, OUTE], F32, kind="ExternalOutput"
    )

    with tile.TileContext(nc) as tc:
        _body(tc, nc, xin, yout)

    nc.compile()
    return nc


def _body(tc, nc, xin, yout):
    from contextlib import ExitStack

    ctx = ExitStack()
    with ctx:
        xpool = ctx.enter_context(tc.tile_pool(name="xpool", bufs=2))
        vpool = ctx.enter_context(tc.tile_pool(name="vpool", bufs=1))
        lmh = ctx.enter_context(tc.tile_pool(name="lmh", bufs=1))
        hpool = ctx.enter_context(tc.tile_pool(name="hpool", bufs=1))
        abc = ctx.enter_context(tc.tile_pool(name="abc", bufs=1))
        fin = ctx.enter_context(tc.tile_pool(name="fin", bufs=1))
        opool = ctx.enter_context(tc.tile_pool(name="opool", bufs=2))

        tt = nc.vector.tensor_tensor

        for t in range(N_PASSES):
            X = xpool.tile([128, SLABE], F32, name="X")
            # chunked input DMA (rows 0..6 then 6..10 of each slab) so the
            # first sub-pass can start while the rest is in flight
            for r0, r1 in ((0, 6), (6, SLAB)):
                nc.sync.dma_start(
                    X[:, r0 * PW : r1 * PW],
                    _dram(
                        xin,
                        t * 128 * SLABE + r0 * PW,
                        [[SLABE, 128], [1, (r1 - r0) * PW]],
                    ),
                )

            for sp in range(2):
                b = sp * SUB  # slab row base for this sub-pass
                def xv(r0, nrows, rstep=2):
                    return _ap(X, (b + r0) * PW, [[rstep * PW, nrows], [1, PW]])

                # ---- vertical stage: column sort3 -> lo, mid, hi ----
                pmin = vpool.tile([128, 2 * PW], F32, name="pmin")
                pmax = vpool.tile([128, 2 * PW], F32, name="pmax")
                pv = [[PW, 2], [1, PW]]
                tt(_ap(pmin, 0, pv), xv(1, 2), xv(2, 2), MIN)
                tt(_ap(pmax, 0, pv), xv(1, 2), xv(2, 2), MAX)

                lo = lmh.tile([128, SUB * PW], F32, name="lo")
                mid = lmh.tile([128, SUB * PW], F32, name="mid")
                hi = lmh.tile([128, SUB * PW], F32, name="hi")
                uo = vpool.tile([128, 2 * PW], F32, name="uo")
                ue = vpool.tile([128, 2 * PW], F32, name="ue")

                def lv(tl, r0):  # rows {r0, r0+2} of a SUB-row tile
                    return _ap(tl, r0 * PW, [[2 * PW, 2], [1, PW]])

                pm = _ap(pmin, 0, pv)
                pM = _ap(pmax, 0, pv)
                tt(lv(lo, 0), pm, xv(0, 2), MIN)
                tt(lv(lo, 1), pm, xv(3, 2), MIN)
                tt(lv(hi, 0), pM, xv(0, 2), MAX)
                tt(lv(hi, 1), pM, xv(3, 2), MAX)
                tt(_ap(uo, 0, pv), pM, xv(0, 2), MIN)
                tt(_ap(ue, 0, pv), pM, xv(3, 2), MIN)
                tt(lv(mid, 0), pm, _ap(uo, 0, pv), MAX)
                tt(lv(mid, 1), pm, _ap(ue, 0, pv), MAX)

                # ---- horizontal stage ----
                NP = PW // 2  # 257 even/odd column pairs
                def cview(tl, c0, ncols, cstep=2):
                    return _ap(tl, c0, [[PW, SUB], [cstep, ncols]])

                def pview(tl, k0, nk):
                    return _ap(tl, k0, [[NP, SUB], [1, nk]])

                mlo = hpool.tile([128, SUB * NP], F32, name="mlo")
                mhi = hpool.tile([128, SUB * NP], F32, name="mhi")
                pmn = hpool.tile([128, SUB * NP], F32, name="pmn")
                pmx = hpool.tile([128, SUB * NP], F32, name="pmx")

                tt(pview(mlo, 0, NP), cview(lo, 0, NP), cview(lo, 1, NP), MAX)
                tt(pview(mhi, 0, NP), cview(hi, 0, NP), cview(hi, 1, NP), MIN)
                tt(pview(pmn, 0, NP), cview(mid, 0, NP), cview(mid, 1, NP), MIN)
                tt(pview(pmx, 0, NP), cview(mid, 0, NP), cview(mid, 1, NP), MAX)

                def ov(tl, c0, ncols, cstep=2):
                    return _ap(tl, c0, [[W, SUB], [cstep, ncols]])

                A = abc.tile([128, SUB * W], F32, name="A")
                Bt = abc.tile([128, SUB * W], F32, name="Bt")
                Ct = abc.tile([128, SUB * W], F32, name="Ct")
                ube = hpool.tile([128, SUB * (W // 2)], F32, name="ube")
                ubo = hpool.tile([128, SUB * (W // 2)], F32, name="ubo")
                NH = W // 2  # 256

                def uv(tl):
                    return _ap(tl, 0, [[NH, SUB], [1, NH]])

                # A = sliding max3 of lo (out col j: even uses pair j/2 +
                # lo[j+2]; odd uses pair (j+1)/2 + lo[j]); C mirrors with min
                tt(ov(A, 0, NH), pview(mlo, 0, NH), cview(lo, 2, NH), MAX)
                tt(ov(A, 1, NH), pview(mlo, 1, NH), cview(lo, 1, NH), MAX)
                tt(ov(Ct, 0, NH), pview(mhi, 0, NH), cview(hi, 2, NH), MIN)
                tt(ov(Ct, 1, NH), pview(mhi, 1, NH), cview(hi, 1, NH), MIN)
                # B = sliding med3 of mid: med3(a, pair) = max(pmn, min(a, pmx))
                tt(uv(ube), cview(mid, 2, NH), pview(pmx, 0, NH), MIN)
                tt(ov(Bt, 0, NH), pview(pmn, 0, NH), uv(ube), MAX)
                tt(uv(ubo), cview(mid, 1, NH), pview(pmx, 1, NH), MIN)
                tt(ov(Bt, 1, NH), pview(pmn, 1, NH), uv(ubo), MAX)

                # ---- final med3(A, B, C) ----
                flat = [[1, OUTE]]
                mn = fin.tile([128, OUTE], F32, name="mn")
                mx = fin.tile([128, OUTE], F32, name="mx")
                t2 = fin.tile([128, OUTE], F32, name="t2")
                res = opool.tile([128, OUTE], F32, name="res")
                tt(_ap(mn, 0, flat), _ap(A, 0, flat), _ap(Bt, 0, flat), MIN)
                tt(_ap(mx, 0, flat), _ap(A, 0, flat), _ap(Bt, 0, flat), MAX)
                tt(_ap(t2, 0, flat), _ap(mx, 0, flat), _ap(Ct, 0, flat), MIN)
                tt(_ap(res, 0, flat), _ap(mn, 0, flat), _ap(t2, 0, flat), MAX)

                nc.sync.dma_start(
                    _dram(
                        yout,
                        (t * 2 + sp) * 128 * OUTE,
                        [[OUTE, 128], [1, OUTE]],
                    ),
                    res[:, :],
                )


def _get_nc():
    if "nc" not in _CACHED:
        _CACHED["nc"] = _build()
    return _CACHED["nc"]


# staged-input row gather: for each chunk c (0..63), padded rows 8c..8c+10
_ROWIDX = (np.arange(64) * CHUNK)[:, None] + np.arange(SLAB)[None, :]


def _stage_input(shard6: np.ndarray) -> np.ndarray:
    """(6, 512, 512) -> [3, 128, SLABE] staged slabs (zero-padded)."""
    padded = np.zeros((PLANES, PH, PW), dtype=np.float32)
    padded[:, 1:-1, 1:-1] = shard6
    # slabs[plane, c] = padded[plane, 8c:8c+10, :]
    slabs = padded[:, _ROWIDX, :]  # (6, 64, 10, 514)
    return slabs.reshape(N_PASSES, 128, SLABE)


def _unstage_output(ys: np.ndarray) -> np.ndarray:
    """[3, 2, 128, OUTE] -> (6, 512, 512)."""
    # ys[t, sp, 64h + c, r*512:...] = plane(2t+h), row 8c + 4sp + r
    arr = ys.reshape(N_PASSES, 2, 2, 64, SUB, W)  # (t, sp, h, c, r, w)
    arr = arr.transpose(0, 2, 3, 1, 4, 5)  # (t, h, c, sp, r, w)
    return arr.reshape(PLANES, H, W)


def kernel(image: np.ndarray, _trace: bool = False):
    assert image.shape == (B, C, H, W) and image.dtype == np.float32
    nc = _get_nc()

    per_core = B // N_CORES
    in_maps = []
    for c in range(N_CORES):
        shard = image[c * per_core : (c + 1) * per_core].reshape(PLANES, H, W)
        in_maps.append({"xs": _stage_input(shard)})

    res = run_bass_kernel_spmd(
        nc, in_maps, list(range(N_CORES)), trace=_trace
    )
    _CACHED["last_exec_ns"] = res.exec_time_ns

    out = np.empty((B, C, H, W), dtype=np.float32)
    for c in range(N_CORES):
        out[c * per_core : (c + 1) * per_core] = _unstage_output(
            res.results[c]["ys"]
        ).reshape(per_core, C, H, W)
    return out


# revision 7
# speedup vs baseline: 1.1388x; 1.0148x over previous
"""3x3 median blur on Trainium2, data-parallel across 8 NeuronCores.

Input:  image (16, 3, 512, 512) float32
Output: median-blur(3x3, zero-padded) same shape.

Strategy:
- Shard batch across 8 cores: core c handles images [2c, 2c+2) -> 6 channel
  planes of 512x512 each.
- Host pads each plane to 514x514 with zeros and restages it into device
  layout [pass][partition][slab]: per pass two planes, each split into 64
  row-chunks of 8 output rows; partition p holds a 10-row x 514-col slab
  (8 output rows + 1 halo row each side). Restaging makes every DMA a plain
  [128 partitions x contiguous] transfer at full SDMA rate.
- Device kernel (per core): 3 passes x 2 sub-passes; exact fp32 median-of-9
  via separable sorting network on the vector engine:
    vertical: sort each 3-tall column into (lo, mid, hi) using shared
      adjacent-row min/max pairs;
    horizontal: median9 = med3(max3(lo), med3(mid), min3(hi)) with shared
      even/odd column pairs.
  ~15 tensor_tensor min/max element-cycles per output pixel, fp32-exact;
  odd/even phases are fused into single instructions via 3-dim access
  patterns (broadcast and negative strides).
- Output staged as [pass][sub-pass][partition][4*512]; host scatters back.
"""

import hashlib
import os
import shutil
import sys

if "/opt/trn_rl_repo" not in sys.path:
    sys.path.insert(0, "/opt/trn_rl_repo")

import numpy as np

import concourse.bass as bass
import concourse.tile as tile
from concourse import bacc, mybir
from concourse.bass_utils import run_bass_kernel_spmd

F32 = mybir.dt.float32
MAX = mybir.AluOpType.max
MIN = mybir.AluOpType.min

N_CORES = 8
B, C, H, W = 16, 3, 512, 512
PLANES = (B * C) // N_CORES  # 6 planes per core
PH, PW = H + 2, W + 2  # 514, 514

N_PASSES = PLANES // 2  # 2 planes per pass
CHUNK = 8  # output rows per partition per pass
SLAB = CHUNK + 2  # input rows per partition slab
SUB = 4  # output rows per sub-pass (2 sub-passes per pass)
SLABE = SLAB * PW  # slab elems per partition (5140)
OUTE = SUB * W  # output elems per partition per sub-pass (2048)
NP = PW // 2  # 257 even/odd column pairs
NH = W // 2  # 256

_CACHED = {}

_NEFF_CACHE_DIR = "/tmp/bass_neff_cache"


def _install_neff_cache():
    """Memoise walrus compiles on disk, keyed by the BIR json hash."""
    if _CACHED.get("neff_cache"):
        return
    import concourse.bass2jax as b2j
    import concourse.bass_utils as bu

    orig = bu.compile_bir_kernel

    def cached_compile(bir_json, tmpdir, neff_name="file.neff"):
        key = hashlib.sha256(bir_json).hexdigest()
        cpath = os.path.join(_NEFF_CACHE_DIR, f"{key}.neff")
        dst = os.path.join(tmpdir, neff_name)
        if os.path.exists(cpath):
            shutil.copy(cpath, dst)
            return dst
        p = orig(bir_json, tmpdir, neff_name)
        try:
            os.makedirs(_NEFF_CACHE_DIR, exist_ok=True)
            tmp = cpath + ".tmp"
            shutil.copy(p, tmp)
            os.replace(tmp, cpath)
        except OSError:
            pass
        return p

    bu.compile_bir_kernel = cached_compile
    b2j.compile_bir_kernel = cached_compile
    _CACHED["neff_cache"] = True


def _ap(apref, off, dims):
    """View into a tile AP with explicit [step, num] free dims."""
    part = list(apref.ap[0])
    return bass.AP(apref.tensor, apref.offset + off, [part] + [list(d) for d in dims])


def _dram(handle, off, dims):
    return bass.AP(handle, off, [list(d) for d in dims])


def _build():
    nc = bacc.Bacc(
        "TRN2", target_bir_lowering=False, debug=False, num_devices=N_CORES
    )
    xin = nc.dram_tensor(
        "xs", [N_PASSES, 128, SLABE], F32, kind="ExternalInput"
    )
    yout = nc.dram_tensor(
        "ys", [N_PASSES, 2, 128, OUTE], F32, kind="ExternalOutput"
    )

    with tile.TileContext(nc) as tc:
        _body(tc, nc, xin, yout)

    nc.compile()
    return nc


def _body(tc, nc, xin, yout):
    from contextlib import ExitStack

    ctx = ExitStack()
    with ctx:
        xpool = ctx.enter_context(tc.tile_pool(name="xpool", bufs=2))
        vpool = ctx.enter_context(tc.tile_pool(name="vpool", bufs=1))
        lmh = ctx.enter_context(tc.tile_pool(name="lmh", bufs=1))
        hpool = ctx.enter_context(tc.tile_pool(name="hpool", bufs=1))
        abc = ctx.enter_context(tc.tile_pool(name="abc", bufs=1))
        fin = ctx.enter_context(tc.tile_pool(name="fin", bufs=1))
        opool = ctx.enter_context(tc.tile_pool(name="opool", bufs=2))

        tt = nc.vector.tensor_tensor

        for t in range(N_PASSES):
            X = xpool.tile([128, SLABE], F32, name="X")
            # Chunked input DMA. Chunk A covers rows 0..6 (everything the
            # first sub-pass reads) plus one element of chunk B's range; the
            # 1-element WAW overlap makes chunk B wait for chunk A, so A runs
            # at full SDMA rate and compute starts as soon as it lands.
            ca_end = 6 * PW + 1
            for e0, e1 in ((0, ca_end), (ca_end - 1, SLABE)):
                nc.sync.dma_start(
                    X[:, e0:e1],
                    _dram(
                        xin, t * 128 * SLABE + e0, [[SLABE, 128], [1, e1 - e0]]
                    ),
                )

            for sp in range(2):
                b = sp * SUB  # slab row base for this sub-pass

                def xv(r0, nrows, rstep=2):
                    return _ap(X, (b + r0) * PW, [[rstep * PW, nrows], [1, PW]])

                # ---- vertical: column sort3 -> lo, mid, hi ----
                # pairs at slab rows (b+1,b+2), (b+3,b+4); fused odd/even
                # sorts via [k=2][pol=2][514] APs: third element is
                # X[b+2k] (pol 0) or X[b+3+2k] (pol 1); output row 2k+pol.
                pmin = vpool.tile([128, 2 * PW], F32, name="pmin")
                pmax = vpool.tile([128, 2 * PW], F32, name="pmax")
                pv = [[PW, 2], [1, PW]]
                tt(_ap(pmin, 0, pv), xv(1, 2), xv(2, 2), MIN)
                tt(_ap(pmax, 0, pv), xv(1, 2), xv(2, 2), MAX)

                lo = lmh.tile([128, SUB * PW], F32, name="lo")
                mid = lmh.tile([128, SUB * PW], F32, name="mid")
                hi = lmh.tile([128, SUB * PW], F32, name="hi")
                u = vpool.tile([128, SUB * PW], F32, name="u")

                vout = [[2 * PW, 2], [PW, 2], [1, PW]]  # row 2k+pol
                vbcast = [[PW, 2], [0, 2], [1, PW]]  # pair k, pol-broadcast
                third = _ap(X, b * PW, [[2 * PW, 2], [3 * PW, 2], [1, PW]])
                pm = _ap(pmin, 0, vbcast)
                pM = _ap(pmax, 0, vbcast)
                tt(_ap(lo, 0, vout), pm, third, MIN)
                tt(_ap(hi, 0, vout), pM, third, MAX)
                tt(_ap(u, 0, vout), pM, third, MIN)
                tt(_ap(mid, 0, vout), pm, _ap(u, 0, vout), MAX)

                # ---- horizontal ----
                def cview(tl, c0, ncols, cstep=2):
                    return _ap(tl, c0, [[PW, SUB], [cstep, ncols]])

                def pview(tl, k0, nk):
                    return _ap(tl, k0, [[NP, SUB], [1, nk]])

                mlo = hpool.tile([128, SUB * NP], F32, name="mlo")
                mhi = hpool.tile([128, SUB * NP], F32, name="mhi")
                pmn = hpool.tile([128, SUB * NP], F32, name="pmn")
                pmx = hpool.tile([128, SUB * NP], F32, name="pmx")

                tt(pview(mlo, 0, NP), cview(lo, 0, NP), cview(lo, 1, NP), MAX)
                tt(pview(mhi, 0, NP), cview(hi, 0, NP), cview(hi, 1, NP), MIN)
                tt(pview(pmn, 0, NP), cview(mid, 0, NP), cview(mid, 1, NP), MIN)
                tt(pview(pmx, 0, NP), cview(mid, 0, NP), cview(mid, 1, NP), MAX)

                # fused sliding windows over output col j = pol + 2*j2:
                #   pair index k = pol + j2, third col = 2 - pol + 2*j2
                A = abc.tile([128, SUB * W], F32, name="A")
                Bt = abc.tile([128, SUB * W], F32, name="Bt")
                Ct = abc.tile([128, SUB * W], F32, name="Ct")
                u2 = hpool.tile([128, SUB * 2 * NH], F32, name="u2")

                hout = [[W, SUB], [1, 2], [2, NH]]
                hpair = lambda tl: _ap(tl, 0, [[NP, SUB], [1, 2], [1, NH]])
                hthird = lambda tl: _ap(tl, 2, [[PW, SUB], [-1, 2], [2, NH]])
                u2v = _ap(u2, 0, [[2 * NH, SUB], [NH, 2], [1, NH]])

                tt(_ap(A, 0, hout), hpair(mlo), hthird(lo), MAX)
                tt(_ap(Ct, 0, hout), hpair(mhi), hthird(hi), MIN)
                tt(u2v, hpair(pmx), hthird(mid), MIN)
                tt(_ap(Bt, 0, hout), hpair(pmn), u2v, MAX)

                # ---- final med3(A, B, C) ----
                flat = [[1, OUTE]]
                mn = fin.tile([128, OUTE], F32, name="mn")
                mx = fin.tile([128, OUTE], F32, name="mx")
                t2 = fin.tile([128, OUTE], F32, name="t2")
                res = opool.tile([128, OUTE], F32, name="res")
                tt(_ap(mn, 0, flat), _ap(A, 0, flat), _ap(Bt, 0, flat), MIN)
                tt(_ap(mx, 0, flat), _ap(A, 0, flat), _ap(Bt, 0, flat), MAX)
                tt(_ap(t2, 0, flat), _ap(mx, 0, flat), _ap(Ct, 0, flat), MIN)
                tt(_ap(res, 0, flat), _ap(mn, 0, flat), _ap(t2, 0, flat), MAX)

                nc.sync.dma_start(
                    _dram(
                        yout,
                        (t * 2 + sp) * 128 * OUTE,
                        [[OUTE, 128], [1, OUTE]],
                    ),
                    res[:, :],
                )


def _get_nc():
    if "nc" not in _CACHED:
        _install_neff_cache()
        _CACHED["nc"] = _build()
    return _CACHED["nc"]


# staged-input row gather: for each chunk c (0..63), padded rows 8c..8c+10
_ROWIDX = (np.arange(64) * CHUNK)[:, None] + np.arange(SLAB)[None, :]


def _stage_input(shard6: np.ndarray) -> np.ndarray:
    """(6, 512, 512) -> [3, 128, SLABE] staged slabs (zero-padded)."""
    padded = np.zeros((PLANES, PH, PW), dtype=np.float32)
    padded[:, 1:-1, 1:-1] = shard6
    slabs = padded[:, _ROWIDX, :]  # (6, 64, 10, 514)
    return slabs.reshape(N_PASSES, 128, SLABE)


def _unstage_output(ys: np.ndarray) -> np.ndarray:
    """[3, 2, 128, OUTE] -> (6, 512, 512)."""
    # ys[t, sp, 64h + c, r*512:...] = plane(2t+h), row 8c + 4sp + r
    arr = ys.reshape(N_PASSES, 2, 2, 64, SUB, W)  # (t, sp, h, c, r, w)
    arr = arr.transpose(0, 2, 3, 1, 4, 5)  # (t, h, c, sp, r, w)
    return arr.reshape(PLANES, H, W)


def kernel(image: np.ndarray, _trace: bool = False):
    assert image.shape == (B, C, H, W) and image.dtype == np.float32
    nc = _get_nc()

    per_core = B // N_CORES
    in_maps = []
    for c in range(N_CORES):
        shard = image[c * per_core : (c + 1) * per_core].reshape(PLANES, H, W)
        in_maps.append({"xs": _stage_input(shard)})

    res = run_bass_kernel_spmd(
        nc, in_maps, list(range(N_CORES)), trace=_trace
    )
    _CACHED["last_exec_ns"] = res.exec_time_ns

    out = np.empty((B, C, H, W), dtype=np.float32)
    for c in range(N_CORES):
        out[c * per_core : (c + 1) * per_core] = _unstage_output(
            res.results[c]["ys"]
        ).reshape(per_core, C, H, W)
    return out
